# revision 24
# baseline (speedup 1.0000x reference)
"""Trainium2 Bass kernel for one GPT-style transformer block (bf16 rework).

Problem: x[8,1024,1024]; per-core = one batch element (data-parallel over 8
NeuronCores).  Per core:
    h1 = LN(x); qkv = h1@Wqkv+b; causal MHA (16 heads, d=64);
    r1 = x + attn@Wproj+b; h2 = LN(r1); out = r1 + relu(h2@W1+b1)@W2+b2

Key design points (v2):
  - Host does x/out transposes (kernel works feature-major end to end) and
    folds the LN affine (g,b) into Wqkv/W1 + biases, so the device LN is just
    (x-mu)*rsqrt(var+eps).
  - Everything on-chip is bf16 except PSUM accumulation and small stat rows:
    halves DMA + SBUF, doubles DVE throughput, enables FWL weight loads.
  - Attention: S^T tiles for both head-parities of a head-block land in one
    [128,1024] two-bank PSUM tile (row-tiled concurrent matmuls), one Exp
    ACTIVATE covers both, causal masking via in-place gpsimd affine_select,
    softmax denominators via a ones-column in the augmented V (row 64 of the
    PV psum).  1/d via DVE reciprocal_approx_fast, broadcast with a rank-1
    matmul, applied by DVE on PV eviction.
  - LN inv-std via exp(-0.5*ln(var+eps)) so the whole kernel uses one ACT
    table set (natural_log_exp_and_others) - no table switches.
  - LN2 stats are interleaved into the proj loop (t-outer) to keep PE busy.
"""

import math
import sys

import numpy as np

sys.path.insert(0, "/opt/trn_rl_repo")

from contextlib import ExitStack

import concourse.bass as bass
import concourse.mybir as mybir
import concourse.tile as tile
from concourse import bacc
from concourse.bass import ts
from concourse.masks import make_identity

F32 = mybir.dt.float32
BF16 = mybir.dt.bfloat16
AF = mybir.ActivationFunctionType

B, T, C, H = 8, 1024, 1024, 16
D = C // H
FF = 4 * C
P = 128
NCH = C // P          # 8 feature chunks
NT = T // P           # 8 token chunks of 128
NQ = T // 512         # 2 query chunks of 512
SCALE = 1.0 / math.sqrt(3 * C // H)
EPS = 1e-5


def _build():
    nc = bacc.Bacc("TRN2", target_bir_lowering=False, debug=False)

    xT_d = nc.dram_tensor("xT", [C, T], BF16, kind="ExternalInput").ap()
    Wqkv_d = nc.dram_tensor("Wqkv", [C, 3 * C], BF16, kind="ExternalInput").ap()
    bqkv_d = nc.dram_tensor("bqkv", [3 * C], F32, kind="ExternalInput").ap()
    Wproj_d = nc.dram_tensor("Wproj", [C, C], BF16, kind="ExternalInput").ap()
    bproj_d = nc.dram_tensor("bproj", [C], F32, kind="ExternalInput").ap()
    W1_d = nc.dram_tensor("W1", [C, FF], BF16, kind="ExternalInput").ap()
    b1_d = nc.dram_tensor("b1", [FF], F32, kind="ExternalInput").ap()
    W2_d = nc.dram_tensor("W2", [FF, C], BF16, kind="ExternalInput").ap()
    b2_d = nc.dram_tensor("b2", [C], F32, kind="ExternalInput").ap()
    outT_d = nc.dram_tensor("outT", [C, T], BF16, kind="ExternalOutput").ap()

    Wqkv_r = Wqkv_d.rearrange("(j p) m -> p j m", p=P)     # [128, 8, 3072]
    Wproj_r = Wproj_d.rearrange("(j p) m -> p j m", p=P)   # [128, 8, 1024]
    W1_r = W1_d.rearrange("(j p) m -> p j m", p=P)         # [128, 8, 4096]
    W2_r = W2_d.rearrange("(j p) m -> p j m", p=P)         # [128, 32, 1024]

    with nc.allow_low_precision(reason="bf16 activations/weights"), \
         tile.TileContext(nc) as tc, ExitStack() as ctx:
        const = ctx.enter_context(tc.tile_pool(name="const", bufs=1))
        xpool = ctx.enter_context(tc.tile_pool(name="xpool", bufs=8))
        hpool = ctx.enter_context(tc.tile_pool(name="hpool", bufs=8))
        qkvp = ctx.enter_context(tc.tile_pool(name="qkvp", bufs=6))
        vaugp = ctx.enter_context(tc.tile_pool(name="vaugp", bufs=16))
        ptp = ctx.enter_context(tc.tile_pool(name="ptp", bufs=3))
        ypool = ctx.enter_context(tc.tile_pool(name="ypool", bufs=8))
        a1pool = ctx.enter_context(tc.tile_pool(name="a1pool", bufs=17))
        wpool = ctx.enter_context(tc.tile_pool(name="wpool", bufs=2))
        spool = ctx.enter_context(tc.tile_pool(name="spool", bufs=2))
        ps_st = ctx.enter_context(tc.tile_pool(name="ps_st", bufs=2, space="PSUM"))
        ps_pv = ctx.enter_context(tc.tile_pool(name="ps_pv", bufs=2, space="PSUM"))
        ps_lin = ctx.enter_context(tc.tile_pool(name="ps_lin", bufs=2, space="PSUM"))

        # ---- load x first (feature-major straight from DRAM); t0 halves
        # first so LN1 stats can start before the full tensor lands
        x_t = [xpool.tile([P, T], BF16, tag="x", name=f"x_fm{m}") for m in range(NCH)]
        for t in range(NQ):
            for m in range(NCH):
                nc.sync.dma_start(x_t[m][:, ts(t, 512)],
                                  xT_d[ts(m, P), ts(t, 512)])

        # ---- constants -------------------------------------------------
        identf = const.tile([P, P], F32)
        make_identity(nc, identf[:])
        identb = const.tile([P, P], BF16)
        nc.scalar.activation(identb[:], identf[:], AF.Copy)
        ones_col = const.tile([P, 1], BF16)
        nc.vector.memset(ones_col[:], 1.0)
        ones_row = const.tile([1, P], BF16)
        nc.vector.memset(ones_row[:], 1.0)
        eps_t = const.tile([1, 1], F32)
        nc.vector.memset(eps_t[:], EPS)

        # bias/param columns: col m = vec[m*128:(m+1)*128]
        bqkv_t = const.tile([P, 3 * NCH], F32)
        nc.sync.dma_start(bqkv_t[:], bqkv_d.rearrange("(m p) -> p m", p=P))
        bproj_t = const.tile([P, NCH], F32)
        nc.sync.dma_start(bproj_t[:], bproj_d.rearrange("(m p) -> p m", p=P))
        b1_t = const.tile([P, FF // P], F32)
        nc.sync.dma_start(b1_t[:], b1_d.rearrange("(m p) -> p m", p=P))
        b2_t = const.tile([P, NCH], F32)
        nc.sync.dma_start(b2_t[:], b2_d.rearrange("(m p) -> p m", p=P))

        def ln_stat_chain(sum_ps, sq_ps, t, name):
            """From accumulated sum/sumsq psum rows produce m2 [1,2,512] bf16:
            slot 0 = inv = (var+eps)^-1/2, slot 1 = -mu*inv."""
            mu = spool.tile([1, 512], F32, tag="stat", bufs=6, name=f"{name}_mu{t}")
            nc.scalar.mul(mu[:], sum_ps[:], 1.0 / C)
            m2e = spool.tile([1, 512], F32, tag="stat", bufs=6, name=f"{name}_m2e{t}")
            nc.scalar.mul(m2e[:], sq_ps[:], 1.0 / C)
            musq = spool.tile([1, 512], F32, tag="stat", bufs=6, name=f"{name}_musq{t}")
            nc.vector.tensor_mul(musq[:], mu[:], mu[:])
            var = spool.tile([1, 512], F32, tag="stat", bufs=6, name=f"{name}_var{t}")
            nc.vector.tensor_sub(var[:], m2e[:], musq[:])
            lg = spool.tile([1, 512], F32, tag="stat", bufs=6, name=f"{name}_lg{t}")
            nc.scalar.activation(lg[:], var[:], AF.Ln, bias=eps_t[:])
            m2 = spool.tile([1, 2, 512], BF16, tag="m2", bufs=4, name=f"{name}_m2{t}")
            nc.scalar.activation(m2[0:1, 0, :], lg[:], AF.Exp, scale=-0.5)
            mmi = spool.tile([1, 512], F32, tag="stat", bufs=6, name=f"{name}_mmi{t}")
            nc.vector.tensor_mul(mmi[:], mu[:], m2[0:1, 0, :])
            nc.scalar.mul(m2[0:1, 1, :], mmi[:], -1.0)
            return m2

        def ln_broadcast(m2, t, name):
            """Materialize inv/c0 rows broadcast across partitions (bf16)."""
            outs = []
            for r, nm in ((0, "inv"), (1, "c0")):
                bps = ps_lin.tile([P, 512], F32, tag="lin", name=f"{name}_b{nm}{t}")
                nc.tensor.matmul(bps[:], ones_row[:], m2[0:1, r, :],
                                 start=True, stop=True)
                bc = spool.tile([P, 512], BF16, tag="lnbc", bufs=4,
                                name=f"{name}_{nm}b{t}")
                nc.vector.tensor_copy(bc[:], bps[:])
                outs.append(bc)
            return outs

        # ---- LN1 (both t stat passes first, chains overlap) ------------
        h1 = [hpool.tile([P, T], BF16, tag="h", name=f"h1_{c}") for c in range(NCH)]
        ln1_stats = []
        for t in range(NQ):
            sum_ps = ps_pv.tile([1, 512], F32, tag="pv", name=f"ln1_sum{t}")
            sq_ps = ps_pv.tile([1, 512], F32, tag="pv", name=f"ln1_sq{t}")
            for c in range(NCH):
                sq = spool.tile([P, 512], BF16, tag="sq", bufs=2,
                                name=f"ln1_sq{c}_{t}")
                nc.vector.tensor_mul(sq[:], x_t[c][:, ts(t, 512)],
                                     x_t[c][:, ts(t, 512)])
                nc.tensor.matmul(sum_ps[:], ones_col[:], x_t[c][:, ts(t, 512)],
                                 start=(c == 0), stop=(c == NCH - 1))
                nc.tensor.matmul(sq_ps[:], ones_col[:], sq[:],
                                 start=(c == 0), stop=(c == NCH - 1))
            ln1_stats.append((sum_ps, sq_ps))
        for t in range(NQ):
            sum_ps, sq_ps = ln1_stats[t]
            m2 = ln_stat_chain(sum_ps, sq_ps, t, "ln1")
            invb, c0b = ln_broadcast(m2, t, "ln1")
            for c in range(NCH):
                nc.vector.tensor_mul(h1[c][:, ts(t, 512)],
                                     x_t[c][:, ts(t, 512)], invb[:])
                nc.vector.tensor_add(h1[c][:, ts(t, 512)],
                                     h1[c][:, ts(t, 512)], c0b[:])

        # bproj pre-add (after LN1 consumed x); r1 = (x + bproj) + attn@Wproj
        for m in range(NCH):
            nc.vector.tensor_scalar_add(x_t[m][:], x_t[m][:],
                                        bproj_t[:, m:m + 1])

        # ---- per-head-block QKV + attention ---------------------------
        y_t = [ypool.tile([P, T], BF16, tag="y", name=f"y{hb}")
               for hb in range(NCH)]

        def dn_finish(hb, qi, yu, dnr):
            """Deferred softmax-denominator normalize: y = yu * (1/d)."""
            dni = spool.tile([1, 2, 512], F32, tag="dn", bufs=4,
                             name=f"dni{hb}_{qi}")
            nc.vector.reciprocal_approx_fast(dni[:], dnr[:])
            dnib = spool.tile([1, 2, 512], BF16, tag="dnb16", bufs=2,
                              name=f"dnib{hb}_{qi}")
            nc.vector.tensor_copy(dnib[:], dni[:])
            bps = ps_lin.tile([P, 512], F32, tag="lin", name=f"dnb{hb}_{qi}")
            for p_ in range(2):
                nc.tensor.matmul(bps[p_ * 64:(p_ + 1) * 64, :],
                                 ones_row[:, 0:64], dnib[0:1, p_, :],
                                 start=True, stop=True)
            for p_ in range(2):
                dnb = spool.tile([64, 512], BF16, tag="dnbb", bufs=4,
                                 name=f"dnbb{hb}_{p_}_{qi}")
                nc.vector.tensor_copy(dnb[:], bps[p_ * 64:(p_ + 1) * 64, :])
                nc.gpsimd.tensor_mul(
                    y_t[hb][p_ * 64:(p_ + 1) * 64, ts(qi, 512)],
                    yu[p_][0:64, :], dnb[:])

        pending = []
        for hb in range(NCH):
            q_t = qkvp.tile([P, T], BF16, tag="qkv", name=f"q{hb}")
            k_t = qkvp.tile([P, T], BF16, tag="qkv", name=f"k{hb}")
            v_t = qkvp.tile([P, T], BF16, tag="qkv", name=f"v{hb}")
            for dst, mcol, ev in ((k_t, NCH + hb, "v"), (q_t, hb, "v"),
                                  (v_t, 2 * NCH + hb, "s")):
                wt = wpool.tile([P, NCH, P], BF16, tag="wqkv", bufs=6,
                                name=f"wqkv{hb}_{mcol}")
                nc.sync.dma_start(wt[:], Wqkv_r[:, :, ts(mcol, P)])
                for t in range(NQ):
                    ps = ps_lin.tile([P, 512], F32, tag="lin",
                                     name=f"qkv_ps{hb}_{mcol}_{t}")
                    for j in range(NCH):
                        nc.tensor.matmul(ps[:], wt[:, j, :],
                                         h1[j][:, ts(t, 512)],
                                         start=(j == 0), stop=(j == NCH - 1))
                    if ev == "v":
                        nc.vector.tensor_scalar_add(dst[:, ts(t, 512)], ps[:],
                                                    bqkv_t[:, mcol:mcol + 1])
                    else:
                        nc.scalar.activation(dst[:, ts(t, 512)], ps[:],
                                             AF.Identity,
                                             bias=bqkv_t[:, mcol:mcol + 1])
            # v -> token-major augmented layout:
            # vaug[ki] = [128(Tk), 130] : cols 0..63 head A, 64 ones,
            #                             65..128 head B, 129 ones
            vaug = [vaugp.tile([P, 130], BF16, tag="vaug", name=f"va{hb}_{ki}")
                    for ki in range(NT)]
            for ki in range(NT):
                pst = ps_lin.tile([P, P], BF16, tag="lin", name=f"vtr{hb}_{ki}")
                nc.tensor.transpose(pst[:], v_t[:, ts(ki, P)], identb[:])
                dst = vaug[ki][:].rearrange("p (h c) -> p h c", h=2)[:, :, 0:64]
                src = pst[:].rearrange("p (h c) -> p h c", h=2)
                nc.vector.tensor_copy(dst, src)
                nc.vector.memset(vaug[ki][:, 64:65], 1.0)
                nc.vector.memset(vaug[ki][:, 129:130], 1.0)
            # finish the previous head-block's softmax normalization here so
            # its matmuls queue behind ready QKV work (no PE head-of-line stall)
            for item in pending:
                dn_finish(*item)
            pending = []
            for qi in range(NQ):
                kmax = 4 * qi + 3
                pv = [ps_pv.tile([65, 512], F32, tag="pv",
                                 name=f"pv{hb}_{p_}_{qi}") for p_ in range(2)]
                for ki in range(kmax + 1):
                    d = ki - 4 * qi  # band offset; <0 for fully-allowed blocks
                    lo = max(0, d) * P  # first causally-reachable column
                    stp = ps_st.tile([P, 2, 512], F32, tag="st",
                                     name=f"st{hb}_{qi}_{ki}")
                    for p_ in range(2):
                        nc.tensor.matmul(
                            stp[:, p_, lo:512],
                            k_t[p_ * 64:(p_ + 1) * 64, ts(ki, P)],
                            q_t[p_ * 64:(p_ + 1) * 64,
                                qi * 512 + lo:(qi + 1) * 512],
                            start=True, stop=True)
                    pt = ptp.tile([P, 2, 512], BF16, tag="pt", bufs=4,
                                  name=f"pt{hb}_{qi}_{ki}")
                    nc.scalar.activation(pt[:, :, lo:512], stp[:, :, lo:512],
                                         AF.Exp, scale=SCALE)
                    if d >= 0:  # diagonal-band block: zero where c < r (local)
                        nc.gpsimd.affine_select(
                            out=pt[:, :, lo:512], in_=pt[:, :, lo:512],
                            pattern=[[0, 2], [1, 512 - lo]],
                            base=0, channel_multiplier=-1,
                            compare_op=mybir.AluOpType.is_ge, fill=0.0)
                    for p_ in range(2):
                        nc.tensor.matmul(
                            pv[p_][:, lo:512],
                            vaug[ki][:, p_ * 65:(p_ + 1) * 65],
                            pt[:, p_, lo:512],
                            start=(ki == 0), stop=(ki == kmax),
                            skip_group_check=True)
                # evict unnormalized PV + denominator row, free psum fast;
                # the reciprocal/broadcast/normalize runs next head-block
                yu = [spool.tile([65, 512], BF16, tag="yu", bufs=6,
                                 name=f"yu{hb}_{p_}_{qi}") for p_ in range(2)]
                dnr = spool.tile([1, 2, 512], F32, tag="dn", bufs=4,
                                 name=f"dnr{hb}_{qi}")
                for p_ in range(2):
                    if qi == 0:
                        # ACT is idle at the qi boundary; frees pv banks fast
                        nc.scalar.copy(yu[p_][:], pv[p_][:])
                    else:
                        nc.vector.tensor_copy(yu[p_][:], pv[p_][:])
                    nc.vector.tensor_copy(dnr[0:1, p_, :], pv[p_][64:65, :])
                pending.append((hb, qi, yu, dnr))
                if hb == NCH - 1 and qi == 0:
                    # no next head-block to hide behind; finish eagerly so
                    # only the last qi's chain is exposed at the proj boundary
                    for item in pending:
                        dn_finish(*item)
                    pending = []

        # prefetch proj weights before the tail dn chain so proj matmuls
        # have no DMA wait behind the last softmax normalization
        wproj_t0 = []
        for m in range(NCH):
            wt = wpool.tile([P, NCH, P], BF16, tag="wproj", bufs=6,
                            name=f"wproj0_{m}")
            nc.sync.dma_start(wt[:], Wproj_r[:, :, ts(m, P)])
            wproj_t0.append(wt)

        # finish the last head-block's softmax normalization
        for item in pending:
            dn_finish(*item)
        pending = []

        # ---- proj + residual + LN2 stats (t-outer; chains hidden) ------
        h2 = [hpool.tile([P, T], BF16, tag="h", name=f"h2_{c}") for c in range(NCH)]
        ln2_stats = []
        for t in range(NQ):
            # t=1 stat rows go to the (idle) st tag so both t coexist
            statp, stag = (ps_pv, "pv") if t == 0 else (ps_st, "st")
            sum_ps = statp.tile([1, 512], F32, tag=stag, name=f"ln2_sum{t}")
            sq_ps = statp.tile([1, 512], F32, tag=stag, name=f"ln2_sq{t}")
            for m in range(NCH):
                if t == 0:
                    wt = wproj_t0[m]
                else:
                    wt = wpool.tile([P, NCH, P], BF16, tag="wproj", bufs=6,
                                    name=f"wproj{t}_{m}")
                    nc.sync.dma_start(wt[:], Wproj_r[:, :, ts(m, P)])
                ps = ps_lin.tile([P, 512], F32, tag="lin", name=f"proj_ps{t}_{m}")
                for j in range(NCH):
                    nc.tensor.matmul(ps[:], wt[:, j, :], y_t[j][:, ts(t, 512)],
                                     start=(j == 0), stop=(j == NCH - 1))
                nc.vector.tensor_add(x_t[m][:, ts(t, 512)],
                                     x_t[m][:, ts(t, 512)], ps[:])
                sq = spool.tile([P, 512], BF16, tag="sq", bufs=2,
                                name=f"ln2_sq{m}_{t}")
                nc.vector.tensor_mul(sq[:], x_t[m][:, ts(t, 512)],
                                     x_t[m][:, ts(t, 512)])
                nc.tensor.matmul(sum_ps[:], ones_col[:], x_t[m][:, ts(t, 512)],
                                 start=(m == 0), stop=(m == NCH - 1))
                nc.tensor.matmul(sq_ps[:], ones_col[:], sq[:],
                                 start=(m == 0), stop=(m == NCH - 1))
            ln2_stats.append((sum_ps, sq_ps))

        def ln2_finish(t):
            m2 = ln_stat_chain(*ln2_stats[t], t, "ln2")
            invb, c0b = ln_broadcast(m2, t, "ln2")
            for c in range(NCH):
                nc.vector.tensor_mul(h2[c][:, ts(t, 512)],
                                     x_t[c][:, ts(t, 512)], invb[:])
                nc.vector.tensor_add(h2[c][:, ts(t, 512)],
                                     h2[c][:, ts(t, 512)], c0b[:])

        ln2_finish(0)  # runs under the proj t=1 matmuls

        # ---- FFN (two d_ff halves) + residual -------------------------
        # half 0 FFN1 runs t=0 first (only needs h2 t=0); the t=1 LN2
        # chain hides under it.  b2 pre-add folds in per (m) after LN2
        # consumed r1.
        a1 = {}
        for mg in range(16):
            a1[mg] = a1pool.tile([P, T], BF16, tag="a1", name=f"a1_{mg}")
            wt = wpool.tile([P, NCH, P], BF16, tag="w1", bufs=5,
                            name=f"w1_{mg}_t0")
            nc.sync.dma_start(wt[:], W1_r[:, :, ts(mg, P)])
            ps = ps_lin.tile([P, 512], F32, tag="lin", name=f"ffn1_ps{mg}_0")
            for j in range(NCH):
                nc.tensor.matmul(ps[:], wt[:, j, :], h2[j][:, 0:512],
                                 start=(j == 0), stop=(j == NCH - 1))
            nc.scalar.activation(a1[mg][:, 0:512], ps[:], AF.Relu,
                                 bias=b1_t[:, mg:mg + 1])
            if mg == 0:
                ln2_finish(1)  # hide the t=1 LN2 chain under FFN1 t=0
        # b2 pre-add (r1 fully consumed by LN2 by now)
        for m in range(NCH):
            nc.vector.tensor_scalar_add(x_t[m][:], x_t[m][:], b2_t[:, m:m + 1])
        for mg in range(16):
            wt = wpool.tile([P, NCH, P], BF16, tag="w1", bufs=5,
                            name=f"w1_{mg}_t1")
            nc.sync.dma_start(wt[:], W1_r[:, :, ts(mg, P)])
            ps = ps_lin.tile([P, 512], F32, tag="lin", name=f"ffn1_ps{mg}_1")
            for j in range(NCH):
                nc.tensor.matmul(ps[:], wt[:, j, :], h2[j][:, 512:1024],
                                 start=(j == 0), stop=(j == NCH - 1))
            nc.scalar.activation(a1[mg][:, 512:1024], ps[:], AF.Relu,
                                 bias=b1_t[:, mg:mg + 1])
        for half in range(2):
            if half == 1:
                for mm_ in range(16):
                    mg = 16 + mm_
                    a1[mg] = a1pool.tile([P, T], BF16, tag="a1", name=f"a1_{mg}")
                    wt = wpool.tile([P, NCH, P], BF16, tag="w1", bufs=5,
                                    name=f"w1_{mg}")
                    nc.sync.dma_start(wt[:], W1_r[:, :, ts(mg, P)])
                    for t in range(NQ):
                        ps = ps_lin.tile([P, 512], F32, tag="lin",
                                         name=f"ffn1_ps{mg}_{t}")
                        for j in range(NCH):
                            nc.tensor.matmul(ps[:], wt[:, j, :],
                                             h2[j][:, ts(t, 512)],
                                             start=(j == 0), stop=(j == NCH - 1))
                        nc.scalar.activation(a1[mg][:, ts(t, 512)], ps[:],
                                             AF.Relu, bias=b1_t[:, mg:mg + 1])
            for m in range(NCH):
                w2t = wpool.tile([P, 16, P], BF16, tag="w2", bufs=2,
                                 name=f"w2_{half}_{m}")
                nc.sync.dma_start(
                    w2t[:], W2_r[:, half * 16:(half + 1) * 16, ts(m, P)])
                for t in range(NQ):
                    ps = ps_lin.tile([P, 512], F32, tag="lin",
                                     name=f"ffn2_ps{half}_{m}_{t}")
                    for j in range(16):
                        nc.tensor.matmul(ps[:], w2t[:, j, :],
                                         a1[half * 16 + j][:, ts(t, 512)],
                                         start=(j == 0), stop=(j == 15))
                    nc.vector.tensor_add(x_t[m][:, ts(t, 512)],
                                         x_t[m][:, ts(t, 512)], ps[:])
                if half == 1:
                    nc.sync.dma_start(outT_d[ts(m, P), :], x_t[m][:])

    nc.compile()
    return nc


_NC_CACHE = {}


def _get_nc():
    if "nc" not in _NC_CACHE:
        _NC_CACHE["nc"] = _build()
    return _NC_CACHE["nc"]


def _make_in_maps(inputs):
    """Host-side prep: fold LN affine into weights, cast to bf16, transpose x."""
    import ml_dtypes

    bf16 = ml_dtypes.bfloat16
    f32 = np.float32
    Wqkv = np.asarray(inputs["Wqkv"], f32)
    W1 = np.asarray(inputs["W1"], f32)
    ln1_g = np.asarray(inputs["ln1_g"], f32)
    ln1_b = np.asarray(inputs["ln1_b"], f32)
    ln2_g = np.asarray(inputs["ln2_g"], f32)
    ln2_b = np.asarray(inputs["ln2_b"], f32)
    shared = {
        "Wqkv": np.ascontiguousarray(Wqkv * ln1_g[:, None]).astype(bf16),
        "bqkv": (np.asarray(inputs["bqkv"], f32) + ln1_b @ Wqkv).astype(f32),
        "Wproj": np.ascontiguousarray(np.asarray(inputs["Wproj"], f32)).astype(bf16),
        "bproj": np.asarray(inputs["bproj"], f32),
        "W1": np.ascontiguousarray(W1 * ln2_g[:, None]).astype(bf16),
        "b1": (np.asarray(inputs["b1"], f32) + ln2_b @ W1).astype(f32),
        "W2": np.ascontiguousarray(np.asarray(inputs["W2"], f32)).astype(bf16),
        "b2": np.asarray(inputs["b2"], f32),
    }
    x = np.asarray(inputs["x"], f32)
    return [dict(shared, xT=np.ascontiguousarray(x[i].T).astype(bf16))
            for i in range(B)]


def kernel(**inputs):
    from concourse.bass_utils import run_bass_kernel_spmd

    nc = _get_nc()
    in_maps = _make_in_maps(inputs)
    res = run_bass_kernel_spmd(nc, in_maps, core_ids=list(range(B)))
    out = np.stack(
        [np.asarray(res.results[i]["outT"], dtype=np.float32).T for i in range(B)],
        axis=0)
    return np.ascontiguousarray(out).astype(np.float32)


# revision 27
# speedup vs baseline: 1.1945x; 1.1945x over previous
"""Trainium2 Bass kernel for one GPT-style transformer block (bf16 rework).

Problem: x[8,1024,1024]; per-core = one batch element (data-parallel over 8
NeuronCores).  Per core:
    h1 = LN(x); qkv = h1@Wqkv+b; causal MHA (16 heads, d=64);
    r1 = x + attn@Wproj+b; h2 = LN(r1); out = r1 + relu(h2@W1+b1)@W2+b2

Key design points (v2):
  - Host does x/out transposes (kernel works feature-major end to end) and
    folds the LN affine (g,b) into Wqkv/W1 + biases, so the device LN is just
    (x-mu)*rsqrt(var+eps).
  - Everything on-chip is bf16 except PSUM accumulation and small stat rows:
    halves DMA + SBUF, doubles DVE throughput, enables FWL weight loads.
  - Attention: S^T tiles for both head-parities of a head-block land in one
    [128,1024] two-bank PSUM tile (row-tiled concurrent matmuls), one Exp
    ACTIVATE covers both, causal masking via in-place gpsimd affine_select,
    softmax denominators via a ones-column in the augmented V (row 64 of the
    PV psum).  1/d via DVE reciprocal_approx_fast, broadcast with a rank-1
    matmul, applied by DVE on PV eviction.
  - LN inv-std via exp(-0.5*ln(var+eps)) so the whole kernel uses one ACT
    table set (natural_log_exp_and_others) - no table switches.
  - LN2 stats are interleaved into the proj loop (t-outer) to keep PE busy.
"""

import math
import sys

import numpy as np

sys.path.insert(0, "/opt/trn_rl_repo")

from contextlib import ExitStack

import concourse.bass as bass
import concourse.mybir as mybir
import concourse.tile as tile
from concourse import bacc
from concourse.bass import ts
from concourse.masks import make_identity

F32 = mybir.dt.float32
BF16 = mybir.dt.bfloat16
AF = mybir.ActivationFunctionType

B, T, C, H = 8, 1024, 1024, 16
D = C // H
FF = 4 * C
P = 128
NCH = C // P          # 8 feature chunks
NT = T // P           # 8 token chunks of 128
NQ = T // 512         # 2 query chunks of 512
SCALE = 1.0 / math.sqrt(3 * C // H)
EPS = 1e-5


def _build():
    nc = bacc.Bacc("TRN2", target_bir_lowering=False, debug=False)

    xT_d = nc.dram_tensor("xT", [C, T], BF16, kind="ExternalInput").ap()
    Wqkv_d = nc.dram_tensor("Wqkv", [C, 3 * C], BF16, kind="ExternalInput").ap()
    bqkv_d = nc.dram_tensor("bqkv", [3 * C], F32, kind="ExternalInput").ap()
    Wproj_d = nc.dram_tensor("Wproj", [C, C], BF16, kind="ExternalInput").ap()
    bproj_d = nc.dram_tensor("bproj", [C], F32, kind="ExternalInput").ap()
    W1_d = nc.dram_tensor("W1", [C, FF], BF16, kind="ExternalInput").ap()
    b1_d = nc.dram_tensor("b1", [FF], F32, kind="ExternalInput").ap()
    W2_d = nc.dram_tensor("W2", [FF, C], BF16, kind="ExternalInput").ap()
    b2_d = nc.dram_tensor("b2", [C], F32, kind="ExternalInput").ap()
    outT_d = nc.dram_tensor("outT", [C, T], BF16, kind="ExternalOutput").ap()

    Wqkv_r = Wqkv_d.rearrange("(j p) m -> p j m", p=P)     # [128, 8, 3072]
    Wproj_r = Wproj_d.rearrange("(j p) m -> p j m", p=P)   # [128, 8, 1024]
    W1_r = W1_d.rearrange("(j p) m -> p j m", p=P)         # [128, 8, 4096]
    W2_r = W2_d.rearrange("(j p) m -> p j m", p=P)         # [128, 32, 1024]

    with nc.allow_low_precision(reason="bf16 activations/weights"), \
         tile.TileContext(nc) as tc, ExitStack() as ctx:
        const = ctx.enter_context(tc.tile_pool(name="const", bufs=1))
        xpool = ctx.enter_context(tc.tile_pool(name="xpool", bufs=8))
        hpool = ctx.enter_context(tc.tile_pool(name="hpool", bufs=8))
        qkvp = ctx.enter_context(tc.tile_pool(name="qkvp", bufs=6))
        vaugp = ctx.enter_context(tc.tile_pool(name="vaugp", bufs=16))
        ptp = ctx.enter_context(tc.tile_pool(name="ptp", bufs=3))
        ypool = ctx.enter_context(tc.tile_pool(name="ypool", bufs=8))
        a1pool = ctx.enter_context(tc.tile_pool(name="a1pool", bufs=17))
        wpool = ctx.enter_context(tc.tile_pool(name="wpool", bufs=2))
        spool = ctx.enter_context(tc.tile_pool(name="spool", bufs=2))
        ps_st = ctx.enter_context(tc.tile_pool(name="ps_st", bufs=2, space="PSUM"))
        ps_pv = ctx.enter_context(tc.tile_pool(name="ps_pv", bufs=2, space="PSUM"))
        ps_lin = ctx.enter_context(tc.tile_pool(name="ps_lin", bufs=2, space="PSUM"))

        # ---- load x first (feature-major straight from DRAM); t0 halves
        # first so LN1 stats can start before the full tensor lands
        x_t = [xpool.tile([P, T], BF16, tag="x", name=f"x_fm{m}") for m in range(NCH)]
        for t in range(NQ):
            for m in range(NCH):
                nc.sync.dma_start(x_t[m][:, ts(t, 512)],
                                  xT_d[ts(m, P), ts(t, 512)])

        # ---- constants -------------------------------------------------
        identf = const.tile([P, P], F32)
        make_identity(nc, identf[:])
        identb = const.tile([P, P], BF16)
        nc.scalar.activation(identb[:], identf[:], AF.Copy)
        ones_col = const.tile([P, 1], BF16)
        nc.vector.memset(ones_col[:], 1.0)
        ones_row = const.tile([1, P], BF16)
        nc.vector.memset(ones_row[:], 1.0)
        eps_t = const.tile([1, 1], F32)
        nc.vector.memset(eps_t[:], EPS)

        # bias/param columns: col m = vec[m*128:(m+1)*128]
        bqkv_t = const.tile([P, 3 * NCH], F32)
        nc.sync.dma_start(bqkv_t[:], bqkv_d.rearrange("(m p) -> p m", p=P))
        bproj_t = const.tile([P, NCH], F32)
        nc.sync.dma_start(bproj_t[:], bproj_d.rearrange("(m p) -> p m", p=P))
        b1_t = const.tile([P, FF // P], F32)
        nc.sync.dma_start(b1_t[:], b1_d.rearrange("(m p) -> p m", p=P))
        b2_t = const.tile([P, NCH], F32)
        nc.sync.dma_start(b2_t[:], b2_d.rearrange("(m p) -> p m", p=P))

        def ln_stat_chain(sum_ps, sq_ps, t, name):
            """From accumulated sum/sumsq psum rows produce m2 [1,2,512] bf16:
            slot 0 = inv = (var+eps)^-1/2, slot 1 = -mu*inv."""
            mu = spool.tile([1, 512], F32, tag="stat", bufs=6, name=f"{name}_mu{t}")
            nc.scalar.mul(mu[:], sum_ps[:], 1.0 / C)
            m2e = spool.tile([1, 512], F32, tag="stat", bufs=6, name=f"{name}_m2e{t}")
            nc.scalar.mul(m2e[:], sq_ps[:], 1.0 / C)
            musq = spool.tile([1, 512], F32, tag="stat", bufs=6, name=f"{name}_musq{t}")
            nc.vector.tensor_mul(musq[:], mu[:], mu[:])
            var = spool.tile([1, 512], F32, tag="stat", bufs=6, name=f"{name}_var{t}")
            nc.vector.tensor_sub(var[:], m2e[:], musq[:])
            lg = spool.tile([1, 512], F32, tag="stat", bufs=6, name=f"{name}_lg{t}")
            nc.scalar.activation(lg[:], var[:], AF.Ln, bias=eps_t[:])
            m2 = spool.tile([1, 2, 512], BF16, tag="m2", bufs=4, name=f"{name}_m2{t}")
            nc.scalar.activation(m2[0:1, 0, :], lg[:], AF.Exp, scale=-0.5)
            mmi = spool.tile([1, 512], F32, tag="stat", bufs=6, name=f"{name}_mmi{t}")
            nc.vector.tensor_mul(mmi[:], mu[:], m2[0:1, 0, :])
            nc.scalar.mul(m2[0:1, 1, :], mmi[:], -1.0)
            return m2

        def ln_broadcast(m2, t, name):
            """Materialize inv/c0 rows broadcast across partitions (bf16)."""
            outs = []
            for r, nm in ((0, "inv"), (1, "c0")):
                bps = ps_lin.tile([P, 512], F32, tag="lin", name=f"{name}_b{nm}{t}")
                nc.tensor.matmul(bps[:], ones_row[:], m2[0:1, r, :],
                                 start=True, stop=True)
                bc = spool.tile([P, 512], BF16, tag="lnbc", bufs=4,
                                name=f"{name}_{nm}b{t}")
                nc.vector.tensor_copy(bc[:], bps[:])
                outs.append(bc)
            return outs

        # ---- LN1 (both t stat passes first, chains overlap) ------------
        h1 = [hpool.tile([P, T], BF16, tag="h", name=f"h1_{c}") for c in range(NCH)]
        ln1_stats = []
        for t in range(NQ):
            sum_ps = ps_pv.tile([1, 512], F32, tag="pv", name=f"ln1_sum{t}")
            sq_ps = ps_pv.tile([1, 512], F32, tag="pv", name=f"ln1_sq{t}")
            for c in range(NCH):
                sq = spool.tile([P, 512], BF16, tag="sq", bufs=2,
                                name=f"ln1_sq{c}_{t}")
                nc.vector.tensor_mul(sq[:], x_t[c][:, ts(t, 512)],
                                     x_t[c][:, ts(t, 512)])
                nc.tensor.matmul(sum_ps[:], ones_col[:], x_t[c][:, ts(t, 512)],
                                 start=(c == 0), stop=(c == NCH - 1))
                nc.tensor.matmul(sq_ps[:], ones_col[:], sq[:],
                                 start=(c == 0), stop=(c == NCH - 1))
            ln1_stats.append((sum_ps, sq_ps))
        for t in range(NQ):
            sum_ps, sq_ps = ln1_stats[t]
            m2 = ln_stat_chain(sum_ps, sq_ps, t, "ln1")
            invb, c0b = ln_broadcast(m2, t, "ln1")
            for c in range(NCH):
                nc.vector.tensor_mul(h1[c][:, ts(t, 512)],
                                     x_t[c][:, ts(t, 512)], invb[:])
                nc.vector.tensor_add(h1[c][:, ts(t, 512)],
                                     h1[c][:, ts(t, 512)], c0b[:])

        # bproj pre-add (after LN1 consumed x); r1 = (x + bproj) + attn@Wproj
        for m in range(NCH):
            nc.vector.tensor_scalar_add(x_t[m][:], x_t[m][:],
                                        bproj_t[:, m:m + 1])

        # ---- per-head-block QKV + attention ---------------------------
        y_t = [ypool.tile([P, T], BF16, tag="y", name=f"y{hb}")
               for hb in range(NCH)]

        def dn_finish(hb, qi, yu, dnr):
            """Deferred softmax-denominator normalize: y = yu * (1/d)."""
            dni = spool.tile([1, 2, 512], F32, tag="dn", bufs=4,
                             name=f"dni{hb}_{qi}")
            nc.vector.reciprocal_approx_fast(dni[:], dnr[:])
            dnib = spool.tile([1, 2, 512], BF16, tag="dnb16", bufs=2,
                              name=f"dnib{hb}_{qi}")
            nc.vector.tensor_copy(dnib[:], dni[:])
            bps = ps_lin.tile([P, 512], F32, tag="lin", name=f"dnb{hb}_{qi}")
            for p_ in range(2):
                nc.tensor.matmul(bps[p_ * 64:(p_ + 1) * 64, :],
                                 ones_row[:, 0:64], dnib[0:1, p_, :],
                                 start=True, stop=True)
            for p_ in range(2):
                dnb = spool.tile([64, 512], BF16, tag="dnbb", bufs=4,
                                 name=f"dnbb{hb}_{p_}_{qi}")
                nc.vector.tensor_copy(dnb[:], bps[p_ * 64:(p_ + 1) * 64, :])
                nc.gpsimd.tensor_mul(
                    y_t[hb][p_ * 64:(p_ + 1) * 64, ts(qi, 512)],
                    yu[p_][0:64, :], dnb[:])

        pending = []
        for hb in range(NCH):
            q_t = qkvp.tile([P, T], BF16, tag="qkv", name=f"q{hb}")
            k_t = qkvp.tile([P, T], BF16, tag="qkv", name=f"k{hb}")
            v_t = qkvp.tile([P, T], BF16, tag="qkv", name=f"v{hb}")
            for dst, mcol, ev in ((k_t, NCH + hb, "v"), (q_t, hb, "v"),
                                  (v_t, 2 * NCH + hb, "s")):
                wt = wpool.tile([P, NCH, P], BF16, tag="wqkv", bufs=6,
                                name=f"wqkv{hb}_{mcol}")
                nc.sync.dma_start(wt[:], Wqkv_r[:, :, ts(mcol, P)])
                for t in range(NQ):
                    ps = ps_lin.tile([P, 512], F32, tag="lin",
                                     name=f"qkv_ps{hb}_{mcol}_{t}")
                    for j in range(NCH):
                        nc.tensor.matmul(ps[:], wt[:, j, :],
                                         h1[j][:, ts(t, 512)],
                                         start=(j == 0), stop=(j == NCH - 1))
                    if ev == "v":
                        nc.vector.tensor_scalar_add(dst[:, ts(t, 512)], ps[:],
                                                    bqkv_t[:, mcol:mcol + 1])
                    else:
                        nc.scalar.activation(dst[:, ts(t, 512)], ps[:],
                                             AF.Identity,
                                             bias=bqkv_t[:, mcol:mcol + 1])
            # v -> token-major augmented layout:
            # vaug[ki] = [128(Tk), 130] : cols 0..63 head A, 64 ones,
            #                             65..128 head B, 129 ones
            vaug = [vaugp.tile([P, 130], BF16, tag="vaug", name=f"va{hb}_{ki}")
                    for ki in range(NT)]
            for ki in range(NT):
                pst = ps_lin.tile([P, P], BF16, tag="lin", name=f"vtr{hb}_{ki}")
                nc.tensor.transpose(pst[:], v_t[:, ts(ki, P)], identb[:])
                dst = vaug[ki][:].rearrange("p (h c) -> p h c", h=2)[:, :, 0:64]
                src = pst[:].rearrange("p (h c) -> p h c", h=2)
                nc.vector.tensor_copy(dst, src)
                nc.vector.memset(vaug[ki][:, 64:65], 1.0)
                nc.vector.memset(vaug[ki][:, 129:130], 1.0)
            # finish the previous head-block's softmax normalization here so
            # its matmuls queue behind ready QKV work (no PE head-of-line stall)
            for item in pending:
                dn_finish(*item)
            pending = []
            for qi in range(NQ):
                kmax = 4 * qi + 3
                pv = [ps_pv.tile([65, 512], F32, tag="pv",
                                 name=f"pv{hb}_{p_}_{qi}") for p_ in range(2)]
                for ki in range(kmax + 1):
                    d = ki - 4 * qi  # band offset; <0 for fully-allowed blocks
                    lo = max(0, d) * P  # first causally-reachable column
                    stp = ps_st.tile([P, 2, 512], F32, tag="st",
                                     name=f"st{hb}_{qi}_{ki}")
                    for p_ in range(2):
                        nc.tensor.matmul(
                            stp[:, p_, lo:512],
                            k_t[p_ * 64:(p_ + 1) * 64, ts(ki, P)],
                            q_t[p_ * 64:(p_ + 1) * 64,
                                qi * 512 + lo:(qi + 1) * 512],
                            start=True, stop=True)
                    pt = ptp.tile([P, 2, 512], BF16, tag="pt", bufs=4,
                                  name=f"pt{hb}_{qi}_{ki}")
                    nc.scalar.activation(pt[:, :, lo:512], stp[:, :, lo:512],
                                         AF.Exp, scale=SCALE)
                    if d >= 0:  # diagonal-band block: zero where c < r (local)
                        nc.gpsimd.affine_select(
                            out=pt[:, :, lo:512], in_=pt[:, :, lo:512],
                            pattern=[[0, 2], [1, 512 - lo]],
                            base=0, channel_multiplier=-1,
                            compare_op=mybir.AluOpType.is_ge, fill=0.0)
                    for p_ in range(2):
                        nc.tensor.matmul(
                            pv[p_][:, lo:512],
                            vaug[ki][:, p_ * 65:(p_ + 1) * 65],
                            pt[:, p_, lo:512],
                            start=(ki == 0), stop=(ki == kmax),
                            skip_group_check=True)
                # evict unnormalized PV + denominator row, free psum fast;
                # the reciprocal/broadcast/normalize runs next head-block
                yu = [spool.tile([65, 512], BF16, tag="yu", bufs=6,
                                 name=f"yu{hb}_{p_}_{qi}") for p_ in range(2)]
                dnr = spool.tile([1, 2, 512], F32, tag="dn", bufs=4,
                                 name=f"dnr{hb}_{qi}")
                for p_ in range(2):
                    nc.vector.tensor_copy(yu[p_][:], pv[p_][:])
                    nc.vector.tensor_copy(dnr[0:1, p_, :], pv[p_][64:65, :])
                pending.append((hb, qi, yu, dnr))
                if hb == NCH - 1 and qi == 0:
                    # no next head-block to hide behind; finish eagerly so
                    # only the last qi's chain is exposed at the proj boundary
                    for item in pending:
                        dn_finish(*item)
                    pending = []

        # prefetch proj weights before the tail dn chain so proj matmuls
        # have no DMA wait behind the last softmax normalization
        wproj_t0 = {}
        for m in range(6):  # = wproj bufs; rest load inline in the proj loop
            wt = wpool.tile([P, NCH, P], BF16, tag="wproj", bufs=6,
                            name=f"wproj0_{m}")
            nc.sync.dma_start(wt[:], Wproj_r[:, :, ts(m, P)])
            wproj_t0[m] = wt

        # finish the last head-block's softmax normalization
        for item in pending:
            dn_finish(*item)
        pending = []

        # ---- proj + residual + LN2 stats (t-outer; chains hidden) ------
        h2 = [hpool.tile([P, T], BF16, tag="h", name=f"h2_{c}") for c in range(NCH)]
        ln2_stats = []
        for t in range(NQ):
            # t=1 stat rows go to the (idle) st tag so both t coexist
            statp, stag = (ps_pv, "pv") if t == 0 else (ps_st, "st")
            sum_ps = statp.tile([1, 512], F32, tag=stag, name=f"ln2_sum{t}")
            sq_ps = statp.tile([1, 512], F32, tag=stag, name=f"ln2_sq{t}")
            for m in range(NCH):
                if t == 0 and m in wproj_t0:
                    wt = wproj_t0[m]
                else:
                    wt = wpool.tile([P, NCH, P], BF16, tag="wproj", bufs=6,
                                    name=f"wproj{t}_{m}")
                    nc.sync.dma_start(wt[:], Wproj_r[:, :, ts(m, P)])
                ps = ps_lin.tile([P, 512], F32, tag="lin", name=f"proj_ps{t}_{m}")
                for j in range(NCH):
                    nc.tensor.matmul(ps[:], wt[:, j, :], y_t[j][:, ts(t, 512)],
                                     start=(j == 0), stop=(j == NCH - 1))
                nc.vector.tensor_add(x_t[m][:, ts(t, 512)],
                                     x_t[m][:, ts(t, 512)], ps[:])
                sq = spool.tile([P, 512], BF16, tag="sq", bufs=2,
                                name=f"ln2_sq{m}_{t}")
                nc.vector.tensor_mul(sq[:], x_t[m][:, ts(t, 512)],
                                     x_t[m][:, ts(t, 512)])
                nc.tensor.matmul(sum_ps[:], ones_col[:], x_t[m][:, ts(t, 512)],
                                 start=(m == 0), stop=(m == NCH - 1))
                nc.tensor.matmul(sq_ps[:], ones_col[:], sq[:],
                                 start=(m == 0), stop=(m == NCH - 1))
            ln2_stats.append((sum_ps, sq_ps))

        def ln2_finish(t):
            m2 = ln_stat_chain(*ln2_stats[t], t, "ln2")
            invb, c0b = ln_broadcast(m2, t, "ln2")
            for c in range(NCH):
                nc.vector.tensor_mul(h2[c][:, ts(t, 512)],
                                     x_t[c][:, ts(t, 512)], invb[:])
                nc.vector.tensor_add(h2[c][:, ts(t, 512)],
                                     h2[c][:, ts(t, 512)], c0b[:])

        ln2_finish(0)  # runs under the proj t=1 matmuls

        # ---- FFN (two d_ff halves) + residual -------------------------
        # half 0 FFN1 runs t=0 first (only needs h2 t=0); the t=1 LN2
        # chain hides under it.  b2 pre-add folds in per (m) after LN2
        # consumed r1.
        a1 = {}
        for mg in range(16):
            a1[mg] = a1pool.tile([P, T], BF16, tag="a1", name=f"a1_{mg}")
            wt = wpool.tile([P, NCH, P], BF16, tag="w1", bufs=5,
                            name=f"w1_{mg}_t0")
            nc.sync.dma_start(wt[:], W1_r[:, :, ts(mg, P)])
            ps = ps_lin.tile([P, 512], F32, tag="lin", name=f"ffn1_ps{mg}_0")
            for j in range(NCH):
                nc.tensor.matmul(ps[:], wt[:, j, :], h2[j][:, 0:512],
                                 start=(j == 0), stop=(j == NCH - 1))
            nc.scalar.activation(a1[mg][:, 0:512], ps[:], AF.Relu,
                                 bias=b1_t[:, mg:mg + 1])
            if mg == 0:
                ln2_finish(1)  # hide the t=1 LN2 chain under FFN1 t=0
        # b2 pre-add (r1 fully consumed by LN2 by now)
        for m in range(NCH):
            nc.vector.tensor_scalar_add(x_t[m][:], x_t[m][:], b2_t[:, m:m + 1])
        for mg in range(16):
            wt = wpool.tile([P, NCH, P], BF16, tag="w1", bufs=5,
                            name=f"w1_{mg}_t1")
            nc.sync.dma_start(wt[:], W1_r[:, :, ts(mg, P)])
            ps = ps_lin.tile([P, 512], F32, tag="lin", name=f"ffn1_ps{mg}_1")
            for j in range(NCH):
                nc.tensor.matmul(ps[:], wt[:, j, :], h2[j][:, 512:1024],
                                 start=(j == 0), stop=(j == NCH - 1))
            nc.scalar.activation(a1[mg][:, 512:1024], ps[:], AF.Relu,
                                 bias=b1_t[:, mg:mg + 1])
        for half in range(2):
            if half == 1:
                for mm_ in range(16):
                    mg = 16 + mm_
                    a1[mg] = a1pool.tile([P, T], BF16, tag="a1", name=f"a1_{mg}")
                    wt = wpool.tile([P, NCH, P], BF16, tag="w1", bufs=5,
                                    name=f"w1_{mg}")
                    nc.sync.dma_start(wt[:], W1_r[:, :, ts(mg, P)])
                    for t in range(NQ):
                        ps = ps_lin.tile([P, 512], F32, tag="lin",
                                         name=f"ffn1_ps{mg}_{t}")
                        for j in range(NCH):
                            nc.tensor.matmul(ps[:], wt[:, j, :],
                                             h2[j][:, ts(t, 512)],
                                             start=(j == 0), stop=(j == NCH - 1))
                        nc.scalar.activation(a1[mg][:, ts(t, 512)], ps[:],
                                             AF.Relu, bias=b1_t[:, mg:mg + 1])
            for m in range(NCH):
                w2t = wpool.tile([P, 16, P], BF16, tag="w2", bufs=2,
                                 name=f"w2_{half}_{m}")
                nc.sync.dma_start(
                    w2t[:], W2_r[:, half * 16:(half + 1) * 16, ts(m, P)])
                for t in range(NQ):
                    ps = ps_lin.tile([P, 512], F32, tag="lin",
                                     name=f"ffn2_ps{half}_{m}_{t}")
                    for j in range(16):
                        nc.tensor.matmul(ps[:], w2t[:, j, :],
                                         a1[half * 16 + j][:, ts(t, 512)],
                                         start=(j == 0), stop=(j == 15))
                    nc.vector.tensor_add(x_t[m][:, ts(t, 512)],
                                         x_t[m][:, ts(t, 512)], ps[:])
                if half == 1:
                    nc.sync.dma_start(outT_d[ts(m, P), :], x_t[m][:])

    nc.compile()
    return nc


_NC_CACHE = {}


def _get_nc():
    if "nc" not in _NC_CACHE:
        _NC_CACHE["nc"] = _build()
    return _NC_CACHE["nc"]


def _make_in_maps(inputs):
    """Host-side prep: fold LN affine into weights, cast to bf16, transpose x."""
    import ml_dtypes

    bf16 = ml_dtypes.bfloat16
    f32 = np.float32
    Wqkv = np.asarray(inputs["Wqkv"], f32)
    W1 = np.asarray(inputs["W1"], f32)
    ln1_g = np.asarray(inputs["ln1_g"], f32)
    ln1_b = np.asarray(inputs["ln1_b"], f32)
    ln2_g = np.asarray(inputs["ln2_g"], f32)
    ln2_b = np.asarray(inputs["ln2_b"], f32)
    shared = {
        "Wqkv": np.ascontiguousarray(Wqkv * ln1_g[:, None]).astype(bf16),
        "bqkv": (np.asarray(inputs["bqkv"], f32) + ln1_b @ Wqkv).astype(f32),
        "Wproj": np.ascontiguousarray(np.asarray(inputs["Wproj"], f32)).astype(bf16),
        "bproj": np.asarray(inputs["bproj"], f32),
        "W1": np.ascontiguousarray(W1 * ln2_g[:, None]).astype(bf16),
        "b1": (np.asarray(inputs["b1"], f32) + ln2_b @ W1).astype(f32),
        "W2": np.ascontiguousarray(np.asarray(inputs["W2"], f32)).astype(bf16),
        "b2": np.asarray(inputs["b2"], f32),
    }
    x = np.asarray(inputs["x"], f32)
    return [dict(shared, xT=np.ascontiguousarray(x[i].T).astype(bf16))
            for i in range(B)]


def kernel(**inputs):
    from concourse.bass_utils import run_bass_kernel_spmd

    nc = _get_nc()
    in_maps = _make_in_maps(inputs)
    res = run_bass_kernel_spmd(nc, in_maps, core_ids=list(range(B)))
    out = np.stack(
        [np.asarray(res.results[i]["outT"], dtype=np.float32).T for i in range(B)],
        axis=0)
    return np.ascontiguousarray(out).astype(np.float32)
